# revision 1
# baseline (speedup 1.0000x reference)
"""Expert-parallel MoE FFN kernel for Trainium2 (8 NeuronCores).

Problem: y[e] = relu(x[e] @ w1[e].T) @ w2[e].T for 8 experts.
Sharding: expert-parallel — expert e runs entirely on core e; no
cross-core communication.

Per-core dataflow (x:[2048,1024], w1:[4096,1024], w2:[1024,4096]):
  phase 0: load x natural, PE-transpose 128x128 blocks -> resident xT
  phase 1: stream w1 rows, PE-transpose -> w1T tiles;
           hT[hid,tok] accumulated in PSUM over d_model tiles, fused
           ReLU on eviction; spill h[4096,2048] to DRAM scratch.
           w2T half 0 (dm cols 0:512) is built during this window so
           the transposes run on a HAM-warm PE and phase 2 can start
           immediately.
  phase 2: stream hT token tiles back; y[tok,dm] accumulated over hid
           tiles against resident w2T. w2T half 1 is built during the
           first token tiles (after xT frees its SBUF); those token
           tiles' second halves are finished in a catch-up loop.

Matmuls run as float32r (full-rate fp32 mode on the PE array); data
stays fp32 end to end; fp32->fp32r rounding happens in the PSUM->SBUF
eviction copies.
"""

import sys

if "/opt/trn_rl_repo" not in sys.path:
    sys.path.insert(0, "/opt/trn_rl_repo")

import numpy as np

import concourse.bass as bass  # noqa: F401
import concourse.mybir as mybir
from concourse import bacc
from concourse.bass_utils import run_bass_kernel_spmd
from concourse.masks import make_identity
from concourse.tile import TileContext

P = 128
TOK = 2048
DM = 1024
DH = 4096
N_CORES = 8

MM_DT = mybir.dt.float32r


def build_nc(mm_dt=MM_DT):
    f32 = mybir.dt.float32
    nc = bacc.Bacc("TRN2", target_bir_lowering=False, debug=False)
    x = nc.dram_tensor("x", [TOK, DM], f32, kind="ExternalInput")
    w1 = nc.dram_tensor("w1", [DH, DM], f32, kind="ExternalInput")
    w2 = nc.dram_tensor("w2", [DM, DH], f32, kind="ExternalInput")
    y = nc.dram_tensor("y", [TOK, DM], f32, kind="ExternalOutput")

    KD = DM // P  # 8 dm tiles (GEMM1 contraction)
    KH = DH // P  # 32 hid tiles (GEMM2 contraction)
    MT = TOK // P  # 16 token tiles
    NB = TOK // 512  # 4 token blocks
    td = mm_dt

    relu = mybir.ActivationFunctionType.Relu
    copyf = mybir.ActivationFunctionType.Copy

    with TileContext(nc) as tc:
        with (
            tc.tile_pool(name="const", bufs=1) as const,
            tc.tile_pool(name="dram", bufs=1, space="DRAM") as dram,
            tc.tile_pool(name="w2T0res", bufs=1) as w2T0_pool,
            tc.tile_pool(name="nat", bufs=6) as nat_pool,
            tc.tile_pool(name="tp", bufs=4, space="PSUM") as tps,
            tc.tile_pool(name="mm", bufs=4, space="PSUM") as mmp,
        ):
            ident = const.tile([P, P], f32)
            make_identity(nc, ident)
            h = dram.tile([DH, TOK], td)

            ncopy = [0]  # alternate PSUM->SBUF copy engine

            def evict_copy(dst, src):
                ncopy[0] += 1
                if ncopy[0] % 2 == 0:
                    nc.vector.tensor_copy(dst, src)
                else:
                    nc.scalar.activation(dst, src, copyf)

            def transpose_block(dst, src):
                pt = tps.tile([P, P], f32, name="tp")
                nc.tensor.transpose(pt[:], src, ident[:])
                evict_copy(dst, pt[:])

            # w2T halves: [P, KH, 512] each (dm cols 0:512 / 512:1024).
            # Half 0 lives for the whole kernel; half 1 is created after
            # xT's pool closes so it reuses that SBUF range.
            w2T = [w2T0_pool.tile([P, KH, 512], td, name="w2T0"), None]

            def build_w2_quarter(dt_, q):
                # transpose w2 rows dt_*128:(dt_+1)*128, hid cols q*1024:(q+1)*1024
                hb = dt_ // 4
                col = (dt_ % 4) * P
                wa = nat_pool.tile([P, DM], f32, tag="nat", name="w2a")
                nc.sync.dma_start(
                    wa[:], w2[dt_ * P : (dt_ + 1) * P, q * DM : (q + 1) * DM]
                )
                for c in range(KD):
                    ht = q * KD + c
                    transpose_block(
                        w2T[hb][:, ht, col : col + P], wa[:, c * P : (c + 1) * P]
                    )

            def build_w2_chunk(dt_):
                for q in range(4):
                    build_w2_quarter(dt_, q)

            # ---- phases 0+1 inside xT's pool scope ----
            p01 = tc.alloc_tile_pool(name="p01", bufs=1)
            w1T_pool = tc.alloc_tile_pool(name="w1T", bufs=2)
            h_pool = tc.alloc_tile_pool(name="hstage", bufs=3)
            xT = p01.tile([P, KD, TOK], td, name="xT")
            for mt in range(MT):
                xa = nat_pool.tile([P, DM], f32, tag="nat", name="xa")
                nc.sync.dma_start(xa[:], x[mt * P : (mt + 1) * P, :])
                for kt in range(KD):
                    transpose_block(
                        xT[:, kt, mt * P : (mt + 1) * P],
                        xa[:, kt * P : (kt + 1) * P],
                    )

            # ---- phase 1: GEMM1 (+ w2T half 0 woven in) ----
            for ht in range(KH):
                wa = nat_pool.tile([P, DM], f32, tag="nat", name="w1a")
                nc.sync.dma_start(wa[:], w1[ht * P : (ht + 1) * P, :])
                w1T = w1T_pool.tile([P, KD, P], td)
                for kt in range(KD):
                    transpose_block(w1T[:, kt, :], wa[:, kt * P : (kt + 1) * P])
                hs = h_pool.tile([P, TOK], td)
                for nb in range(NB):
                    ps = mmp.tile([P, 512], f32, tag="ps", name="ps1")
                    for kt in range(KD):
                        nc.tensor.matmul(
                            ps[:],
                            w1T[:, kt, :],
                            xT[:, kt, nb * 512 : (nb + 1) * 512],
                            start=(kt == 0),
                            stop=(kt == KD - 1),
                        )
                    seg = slice(nb * 512, (nb + 1) * 512)
                    if nb % 2 == 0:
                        nc.scalar.activation(hs[:, seg], ps[:], relu)
                    else:
                        nc.vector.tensor_scalar_max(hs[:, seg], ps[:], 0.0)
                nc.sync.dma_start(h[ht * P : (ht + 1) * P, :], hs[:])
                if ht % 2 == 1:
                    qg = ht // 2  # 0..15 -> dt_ 0..3 (w2T half 0)
                    build_w2_quarter(qg // 4, qg % 4)

            # ---- phase 2: GEMM2 ----
            h_pool.release()
            w1T_pool.release()
            p01.release()
            w2T1_pool = tc.alloc_tile_pool(name="w2T1res", bufs=1)
            hT_pool = tc.alloc_tile_pool(name="hT", bufs=2)
            y_pool = tc.alloc_tile_pool(name="ys", bufs=4)
            w2T[1] = w2T1_pool.tile([P, KH, 512], td, name="w2T1")
            hT_view = h[:].rearrange("(ho p) t -> p ho t", p=P)

            def gemm2_group(mt, db, hTt):
                ps = mmp.tile([P, 512], f32, tag="ps", name="ps2")
                for ht in range(KH):
                    nc.tensor.matmul(
                        ps[:],
                        hTt[:, ht, :],
                        w2T[db][:, ht, :],
                        start=(ht == 0),
                        stop=(ht == KH - 1),
                    )
                ys = y_pool.tile([P, 512], f32)
                evict_copy(ys[:], ps[:])
                nc.sync.dma_start(
                    y[mt * P : (mt + 1) * P, db * 512 : (db + 1) * 512], ys[:]
                )

            def load_hT(mt):
                hTt = hT_pool.tile([P, KH, P], td, name="hTt")
                for hq in range(4):
                    nc.sync.dma_start(
                        hTt[:, hq * 8 : (hq + 1) * 8, :],
                        hT_view[:, hq * 8 : (hq + 1) * 8, mt * P : (mt + 1) * P],
                    )
                return hTt

            for mt in range(MT):
                hTt = load_hT(mt)
                if mt < 4:
                    # build w2T half 1 while GEMM2 starts (xT space frees now)
                    build_w2_chunk(4 + mt)
                gemm2_group(mt, 0, hTt)
                if mt >= 4:
                    gemm2_group(mt, 1, hTt)
            for mt in range(4):  # catch-up: second halves of the first 4 tiles
                hTt = load_hT(mt)
                gemm2_group(mt, 1, hTt)
            y_pool.release()
            hT_pool.release()
            w2T1_pool.release()
    nc.compile()
    return nc


_CACHE = {}


def _get_nc():
    if "nc" not in _CACHE:
        _CACHE["nc"] = build_nc()
    return _CACHE["nc"]


def kernel(x, weight1, weight2):
    x = np.asarray(x, dtype=np.float32)
    weight1 = np.asarray(weight1, dtype=np.float32)
    weight2 = np.asarray(weight2, dtype=np.float32)
    assert x.shape == (N_CORES, TOK, DM)
    assert weight1.shape == (N_CORES, DH, DM)
    assert weight2.shape == (N_CORES, DM, DH)

    nc = _get_nc()
    in_maps = [
        {
            "x": np.ascontiguousarray(x[e]),
            "w1": np.ascontiguousarray(weight1[e]),
            "w2": np.ascontiguousarray(weight2[e]),
        }
        for e in range(N_CORES)
    ]
    res = run_bass_kernel_spmd(nc, in_maps, core_ids=list(range(N_CORES)))
    y = np.stack([res.results[e]["y"] for e in range(N_CORES)], axis=0)
    return y.reshape(1, N_CORES, TOK, DM)



# revision 2
# speedup vs baseline: 1.3332x; 1.3332x over previous
"""Expert-parallel MoE FFN kernel for Trainium2 (8 NeuronCores).

Problem: y[e] = relu(x[e] @ w1[e].T) @ w2[e].T for 8 experts.
Sharding: expert-parallel — expert e runs entirely on core e; no
cross-core communication.

Host-side prep (part of the sharding step): each expert's x, w1, w2 are
transposed to the layouts the PE array consumes (contraction dim on
partitions) and cast to bf16. That removes all 640 on-device PE
transposes (which cost ~120us of tensor-engine time via unhidden
LDWEIGHTS) and cuts DMA-in from 40MB fp32 to 20MB bf16.

Per-core dataflow (xT:[1024,2048], w1T:[1024,4096], w2T:[4096,1024]):
  w1T and w2T live in SBUF for the whole kernel (64 KiB/partition each).
  For each 512-token block:
    GEMM1: hT[dh_tile, tok] accumulated in PSUM over the 8 d_model
           tiles; fused ReLU + bf16 cast on eviction into a resident
           hT[128, 32, 512] tile. No DRAM spill.
    GEMM2: y[tok_tile, dm] accumulated in PSUM over the 32 d_hidden
           tiles against resident w2T; evicted fp32 and DMA'd out.
  All matmuls stream 512 columns (213ns) which hides every LDWEIGHTS;
  tensor-engine work is the 2048 essential GEMM matmuls only.
"""

import sys

if "/opt/trn_rl_repo" not in sys.path:
    sys.path.insert(0, "/opt/trn_rl_repo")

import ml_dtypes
import numpy as np

import concourse.bass as bass  # noqa: F401
import concourse.mybir as mybir
from concourse import bacc
from concourse.bass_utils import run_bass_kernel_spmd
from concourse.tile import TileContext

P = 128
TOK = 2048
DM = 1024
DH = 4096
N_CORES = 8

KD = DM // P  # 8 d_model tiles (GEMM1 contraction)
KH = DH // P  # 32 d_hidden tiles (GEMM2 contraction)
TB = 512  # token block
NTB = TOK // TB  # 4 token blocks
MPB = TB // P  # 4 token tiles per block
QW = DM  # w1 load chunk (dh columns per DMA)

BF16 = ml_dtypes.bfloat16


def build_nc():
    f32 = mybir.dt.float32
    bf = mybir.dt.bfloat16
    nc = bacc.Bacc("TRN2", target_bir_lowering=False, debug=False)
    xT = nc.dram_tensor("xT", [DM, TOK], bf, kind="ExternalInput")
    w1T = nc.dram_tensor("w1T", [DM, DH], bf, kind="ExternalInput")
    w2T = nc.dram_tensor("w2T", [DH, DM], bf, kind="ExternalInput")
    y = nc.dram_tensor("y", [TOK, DM], f32, kind="ExternalOutput")

    relu = mybir.ActivationFunctionType.Relu
    copyf = mybir.ActivationFunctionType.Copy

    with TileContext(nc) as tc:
        with (
            tc.tile_pool(name="w1res", bufs=1) as w1p,
            tc.tile_pool(name="w2res", bufs=1) as w2p,
            tc.tile_pool(name="xt", bufs=NTB) as xp,
            tc.tile_pool(name="ht", bufs=1) as hp,
            tc.tile_pool(name="ys", bufs=4) as yp,
            tc.tile_pool(name="mm", bufs=8, space="PSUM") as mmp,
        ):
            w1t = w1p.tile([P, KD, DH], bf, name="w1t")
            w2t = w2p.tile([P, KH, DM], bf, name="w2t")

            # x token blocks: all DMAs issued up front so none queues
            # behind y write-outs.
            xb = []
            for tb in range(NTB):
                xt = xp.tile([P, KD, TB], bf, name="xt")
                for kt in range(KD):
                    nc.sync.dma_start(
                        xt[:, kt, :],
                        xT[kt * P : (kt + 1) * P, tb * TB : (tb + 1) * TB],
                    )
                xb.append(xt)
                if tb == 0:
                    # first w1 chunk right behind the first x block so
                    # GEMM1 can start ~8us in
                    for kt in range(KD):
                        nc.sync.dma_start(
                            w1t[:, kt, 0:QW],
                            w1T[kt * P : (kt + 1) * P, 0:QW],
                        )
            for q in range(1, DH // QW):
                for kt in range(KD):
                    nc.sync.dma_start(
                        w1t[:, kt, q * QW : (q + 1) * QW],
                        w1T[kt * P : (kt + 1) * P, q * QW : (q + 1) * QW],
                    )
            for ht in range(KH):
                nc.sync.dma_start(w2t[:, ht, :], w2T[ht * P : (ht + 1) * P, :])

            ncopy = [0]  # alternate PSUM->SBUF eviction engine

            def evict(dst, src, do_relu):
                ncopy[0] += 1
                if ncopy[0] % 2 == 0:
                    if do_relu:
                        nc.vector.tensor_scalar_max(dst, src, 0.0)
                    else:
                        nc.vector.tensor_copy(dst, src)
                else:
                    nc.scalar.activation(dst, src, relu if do_relu else copyf)

            for tb in range(NTB):
                hT = hp.tile([P, KH, TB], bf, name="hT")
                # GEMM1: hT[ht, tok] = relu(sum_kt w1T[kt,ht].T @ xT[kt,tok])
                for ht in range(KH):
                    ps = mmp.tile([P, TB], f32, tag="ps", name="ps1")
                    for kt in range(KD):
                        nc.tensor.matmul(
                            ps[:],
                            w1t[:, kt, ht * P : (ht + 1) * P],
                            xb[tb][:, kt, :],
                            start=(kt == 0),
                            stop=(kt == KD - 1),
                        )
                    evict(hT[:, ht, :], ps[:], True)
                # GEMM2: y[mt, dm] = sum_ht hT[ht, mt].T @ w2T[ht, dm]
                for mt in range(MPB):
                    for db in range(2):
                        ps = mmp.tile([P, TB], f32, tag="ps", name="ps2")
                        for ht in range(KH):
                            nc.tensor.matmul(
                                ps[:],
                                hT[:, ht, mt * P : (mt + 1) * P],
                                w2t[:, ht, db * TB : (db + 1) * TB],
                                start=(ht == 0),
                                stop=(ht == KH - 1),
                            )
                        ys = yp.tile([P, TB], f32, name="ys")
                        evict(ys[:], ps[:], False)
                        row = tb * TB + mt * P
                        nc.sync.dma_start(
                            y[row : row + P, db * TB : (db + 1) * TB], ys[:]
                        )
    nc.compile()
    return nc


def make_in_maps(x, weight1, weight2):
    return [
        {
            "xT": x[e].T.astype(BF16),
            "w1T": weight1[e].T.astype(BF16),
            "w2T": weight2[e].T.astype(BF16),
        }
        for e in range(N_CORES)
    ]


_CACHE = {}


def _get_nc():
    if "nc" not in _CACHE:
        _CACHE["nc"] = build_nc()
    return _CACHE["nc"]


def kernel(x, weight1, weight2):
    x = np.asarray(x, dtype=np.float32)
    weight1 = np.asarray(weight1, dtype=np.float32)
    weight2 = np.asarray(weight2, dtype=np.float32)
    assert x.shape == (N_CORES, TOK, DM)
    assert weight1.shape == (N_CORES, DH, DM)
    assert weight2.shape == (N_CORES, DM, DH)

    nc = _get_nc()
    in_maps = make_in_maps(x, weight1, weight2)
    res = run_bass_kernel_spmd(nc, in_maps, core_ids=list(range(N_CORES)))
    y = np.stack([res.results[e]["y"] for e in range(N_CORES)], axis=0)
    return y.reshape(1, N_CORES, TOK, DM)


# revision 4
# speedup vs baseline: 1.3675x; 1.0257x over previous
"""Expert-parallel MoE FFN kernel for Trainium2 (8 NeuronCores).

Problem: y[e] = relu(x[e] @ w1[e].T) @ w2[e].T for 8 experts.
Sharding: expert-parallel — expert e runs entirely on core e; no
cross-core communication.

Host-side prep (part of the sharding step): each expert's x, w1, w2 are
transposed to the layouts the PE array consumes (contraction dim on
partitions) and cast to bf16. That removes all 640 on-device PE
transposes (which cost ~120us of tensor-engine time via unhidden
LDWEIGHTS) and cuts DMA-in from 40MB fp32 to 20MB bf16.

Per-core dataflow (xT:[1024,2048], w1T:[1024,4096], w2T:[4096,1024]):
  w1T and w2T live in SBUF for the whole kernel (64 KiB/partition each).
  For each 512-token block:
    GEMM1: hT[dh_tile, tok] accumulated in PSUM over the 8 d_model
           tiles; fused ReLU + bf16 cast on eviction into a resident
           hT[128, 32, 512] tile. No DRAM spill.
    GEMM2: y[tok_tile, dm] accumulated in PSUM over the 32 d_hidden
           tiles against resident w2T; evicted fp32 and DMA'd out.
  All matmuls stream 512 columns (213ns) which hides every LDWEIGHTS;
  tensor-engine work is the 2048 essential GEMM matmuls only.
"""

import sys

if "/opt/trn_rl_repo" not in sys.path:
    sys.path.insert(0, "/opt/trn_rl_repo")

import ml_dtypes
import numpy as np

import concourse.bass as bass  # noqa: F401
import concourse.mybir as mybir
from concourse import bacc
from concourse.bass_utils import run_bass_kernel_spmd
from concourse.tile import TileContext

P = 128
TOK = 2048
DM = 1024
DH = 4096
N_CORES = 8

KD = DM // P  # 8 d_model tiles (GEMM1 contraction)
KH = DH // P  # 32 d_hidden tiles (GEMM2 contraction)
TB = 512  # token block
NTB = TOK // TB  # 4 token blocks
MPB = TB // P  # 4 token tiles per block
QW = DM  # w1 load chunk (dh columns per DMA)

BF16 = ml_dtypes.bfloat16


def build_nc():
    f32 = mybir.dt.float32
    bf = mybir.dt.bfloat16
    nc = bacc.Bacc("TRN2", target_bir_lowering=False, debug=False)
    xT = nc.dram_tensor("xT", [DM, TOK], bf, kind="ExternalInput")
    w1T = nc.dram_tensor("w1T", [DM, DH], bf, kind="ExternalInput")
    w2T = nc.dram_tensor("w2T", [DH, DM], bf, kind="ExternalInput")
    y = nc.dram_tensor("y", [TOK, DM], f32, kind="ExternalOutput")

    relu = mybir.ActivationFunctionType.Relu
    copyf = mybir.ActivationFunctionType.Copy

    with TileContext(nc) as tc:
        with (
            tc.tile_pool(name="w1res", bufs=1) as w1p,
            tc.tile_pool(name="w2res", bufs=1) as w2p,
            tc.tile_pool(name="xt", bufs=NTB) as xp,
            tc.tile_pool(name="ht", bufs=1) as hp,
            tc.tile_pool(name="ys", bufs=4) as yp,
            tc.tile_pool(name="mm", bufs=8, space="PSUM") as mmp,
        ):
            w1t = w1p.tile([P, KD, DH], bf, name="w1t")
            w2t = w2p.tile([P, KH, DM], bf, name="w2t")

            # Startup: interleave first x block and first w1 chunk per kt
            # so GEMM1's first accumulation group can start as soon as the
            # kt=0 pair lands (DMA issue on the sync engine costs ~630ns
            # per dma_start, so issue order is arrival order).
            xb = [xp.tile([P, KD, TB], bf, name="xt") for _ in range(NTB)]
            for kt in range(KD):
                nc.sync.dma_start(
                    xb[0][:, kt, :], xT[kt * P : (kt + 1) * P, 0:TB]
                )
                nc.sync.dma_start(
                    w1t[:, kt, 0:QW], w1T[kt * P : (kt + 1) * P, 0:QW]
                )
            # Bulk, in consumption order: rest of w1, then w2 (first
            # needed ~55us in), then x blocks 1..3 (first needed ~110us).
            for q in range(1, DH // QW):
                for kt in range(KD):
                    nc.sync.dma_start(
                        w1t[:, kt, q * QW : (q + 1) * QW],
                        w1T[kt * P : (kt + 1) * P, q * QW : (q + 1) * QW],
                    )
            for ht in range(KH):
                nc.sync.dma_start(w2t[:, ht, :], w2T[ht * P : (ht + 1) * P, :])
            for tb in range(1, NTB):
                for kt in range(KD):
                    nc.sync.dma_start(
                        xb[tb][:, kt, :],
                        xT[kt * P : (kt + 1) * P, tb * TB : (tb + 1) * TB],
                    )

            # PE warmup: dummy matmuls with no DMA dependency fill the
            # ~6us window while the first x/w1 chunks land, ramping the
            # tensor engine out of its low p-state before real work.
            wu = w1p.tile([P, TB], bf, name="wu")
            nc.gpsimd.memset(wu[:], 0.0)
            for _ in range(10):
                pw = mmp.tile([P, TB], f32, tag="ps", name="psw")
                nc.tensor.matmul(pw[:], wu[:, 0:P], wu[:], start=True, stop=True)

            ncopy = [0]  # alternate PSUM->SBUF eviction engine

            def evict(dst, src, do_relu):
                ncopy[0] += 1
                if ncopy[0] % 2 == 0:
                    if do_relu:
                        nc.vector.tensor_scalar_max(dst, src, 0.0)
                    else:
                        nc.vector.tensor_copy(dst, src)
                else:
                    nc.scalar.activation(dst, src, relu if do_relu else copyf)

            for tb in range(NTB):
                hT = hp.tile([P, KH, TB], bf, name="hT")
                # GEMM1: hT[ht, tok] = relu(sum_kt w1T[kt,ht].T @ xT[kt,tok])
                for ht in range(KH):
                    ps = mmp.tile([P, TB], f32, tag="ps", name="ps1")
                    for kt in range(KD):
                        nc.tensor.matmul(
                            ps[:],
                            w1t[:, kt, ht * P : (ht + 1) * P],
                            xb[tb][:, kt, :],
                            start=(kt == 0),
                            stop=(kt == KD - 1),
                        )
                    evict(hT[:, ht, :], ps[:], True)
                # GEMM2: y[mt, dm] = sum_ht hT[ht, mt].T @ w2T[ht, dm]
                for mt in range(MPB):
                    for db in range(2):
                        ps = mmp.tile([P, TB], f32, tag="ps", name="ps2")
                        for ht in range(KH):
                            nc.tensor.matmul(
                                ps[:],
                                hT[:, ht, mt * P : (mt + 1) * P],
                                w2t[:, ht, db * TB : (db + 1) * TB],
                                start=(ht == 0),
                                stop=(ht == KH - 1),
                            )
                        ys = yp.tile([P, TB], f32, name="ys")
                        evict(ys[:], ps[:], False)
                        row = tb * TB + mt * P
                        nc.sync.dma_start(
                            y[row : row + P, db * TB : (db + 1) * TB], ys[:]
                        )
    nc.compile()
    return nc


def make_in_maps(x, weight1, weight2):
    return [
        {
            "xT": x[e].T.astype(BF16),
            "w1T": weight1[e].T.astype(BF16),
            "w2T": weight2[e].T.astype(BF16),
        }
        for e in range(N_CORES)
    ]


_CACHE = {}


def _get_nc():
    if "nc" not in _CACHE:
        _CACHE["nc"] = build_nc()
    return _CACHE["nc"]


def kernel(x, weight1, weight2):
    x = np.asarray(x, dtype=np.float32)
    weight1 = np.asarray(weight1, dtype=np.float32)
    weight2 = np.asarray(weight2, dtype=np.float32)
    assert x.shape == (N_CORES, TOK, DM)
    assert weight1.shape == (N_CORES, DH, DM)
    assert weight2.shape == (N_CORES, DM, DH)

    nc = _get_nc()
    in_maps = make_in_maps(x, weight1, weight2)
    res = run_bass_kernel_spmd(nc, in_maps, core_ids=list(range(N_CORES)))
    y = np.stack([res.results[e]["y"] for e in range(N_CORES)], axis=0)
    return y.reshape(1, N_CORES, TOK, DM)
